# revision 4
# baseline (speedup 1.0000x reference)
"""AUROC surrogate loss kernel for 8 TRN2 NeuronCores.

Reference computes, for s = sigmoid(y_pred), pos = (y_true==1), neg = (y_true==0):
    total = sum_{i in pos, j in neg} relu(1 - (s_i - s_j))
    loss  = total / (P*Q)   (0 if either class empty)

Because s in [0, 1], we have s_i - s_j in [-1, 1], so 1 - (s_i - s_j) >= 0 and
the relu never clips.  The pairwise sum is therefore exactly linear:
    total = P*Q - Q*S_pos + P*S_neg
    loss  = 1 - S_pos/P + S_neg/Q
with S_pos = sum of s over positives, S_neg over negatives.  This turns the
O(N^2) pairwise problem into an O(N) streaming reduction (memory-bound).

Sharding: data-parallel -- each of the 8 cores reduces a contiguous 2048-element
shard to per-partition partial sums [128, 3] = (sum s, sum s*t, sum t); the host
gathers the tiny partials and applies the closed-form final formula.
"""

import numpy as np

N = 16384
N_CORES = 8
SHARD = N // N_CORES  # 2048
P = 128
F = SHARD // P  # 16

_NC_CACHE = {}


def _build_nc():
    import concourse.bacc as bacc
    import concourse.tile as tile
    from concourse import mybir

    nc = bacc.Bacc(num_devices=N_CORES)
    yp = nc.dram_tensor("y_pred", [SHARD], mybir.dt.float32, kind="ExternalInput")
    yt = nc.dram_tensor("y_true", [SHARD], mybir.dt.int32, kind="ExternalInput")
    out = nc.dram_tensor("partials", [P, 3], mybir.dt.float32, kind="ExternalOutput")

    yp2 = yp.ap().rearrange("(p f) -> p f", p=P)
    yt2 = yt.ap().rearrange("(p f) -> p f", p=P)

    with tile.TileContext(nc) as tc:
        with tc.tile_pool(name="pool", bufs=1) as pool:
            ypt = pool.tile([P, F], mybir.dt.float32)
            ytt = pool.tile([P, F], mybir.dt.int32)
            nc.sync.dma_start(out=ypt, in_=yp2)
            nc.sync.dma_start(out=ytt, in_=yt2)

            s = pool.tile([P, F], mybir.dt.float32)
            tf = pool.tile([P, F], mybir.dt.float32)
            prod = pool.tile([P, F], mybir.dt.float32)
            red = pool.tile([P, 3], mybir.dt.float32)

            # s = sigmoid(y_pred); red[:,0] = per-partition sum of s
            nc.scalar.activation(
                out=s,
                in_=ypt,
                func=mybir.ActivationFunctionType.Sigmoid,
                accum_out=red[:, 0:1],
            )
            # t as f32
            nc.vector.tensor_copy(out=tf, in_=ytt)
            # red[:,1] = per-partition sum of s*t
            nc.vector.tensor_mul(out=prod, in0=s, in1=tf)
            nc.vector.reduce_sum(out=red[:, 1:2], in_=prod, axis=mybir.AxisListType.X)
            # red[:,2] = per-partition sum of t
            nc.vector.reduce_sum(out=red[:, 2:3], in_=tf, axis=mybir.AxisListType.X)

            nc.sync.dma_start(out=out.ap(), in_=red)
    nc.compile()
    return nc


def get_nc():
    if "nc" not in _NC_CACHE:
        _NC_CACHE["nc"] = _build_nc()
    return _NC_CACHE["nc"]


def _finalize(partials):
    """partials: [n_cores, 128, 3] f32 -> scalar f32 loss."""
    tot = partials.reshape(-1, 3).sum(axis=0, dtype=np.float64)
    s_all, s_pos, p_cnt = tot
    q_cnt = float(N) - p_cnt
    if p_cnt * q_cnt <= 0:
        return np.array(0.0, dtype=np.float32)
    s_neg = s_all - s_pos
    loss = 1.0 - s_pos / p_cnt + s_neg / q_cnt
    return np.array(loss, dtype=np.float32)


def kernel(y_pred, y_true):
    from concourse import bass_utils

    y_pred = np.asarray(y_pred, dtype=np.float32).reshape(N)
    y_true = np.asarray(y_true, dtype=np.int32).reshape(N)

    nc = get_nc()
    in_maps = [
        {
            "y_pred": np.ascontiguousarray(y_pred[i * SHARD : (i + 1) * SHARD]),
            "y_true": np.ascontiguousarray(y_true[i * SHARD : (i + 1) * SHARD]),
        }
        for i in range(N_CORES)
    ]
    res = bass_utils.run_bass_kernel_spmd(nc, in_maps, core_ids=list(range(N_CORES)))
    partials = np.stack([r["partials"] for r in res.results])
    return _finalize(partials)
